# revision 1
# baseline (speedup 1.0000x reference)
"""Trainium2 Bass kernel for nn_CrossAttention (b,m,c,H,W cross-attention).

Problem (hardcoded shapes): b=1, m=4, n=3, c=64, H=W=32, heads=8, dim_head=32.

  q  = Wq  @ x1   per frame        (256, 1024)
  kv = Wkv @ x2   per frame        (512, 3072)
  per (frame, head): attn softmax((q k^T)/sqrt(d)) @ v,  d=32
  y  = Wout @ out  per frame       (64, 1024)

Sharding: 8 cores = 4 frames x 2 q-token halves. Each core gets all 8 heads,
512 q tokens, the full 3072 kv tokens of its frame. No cross-core comms;
outputs concatenate.

Per-core kernel layout strategy:
  - QT (256, 512) / KT (256, 3072) channel-major from 1x1-conv projections,
    heads at 32-partition offsets (quad tiles of 4 heads x 32 dims = 128).
  - scores computed TRANSPOSED: ST[j, i] = sum_d k[j,d] q[i,d] via PE
    row-tiling (4 heads concurrently, K=32 each at tile_position (32r, 0)).
  - softmax without max-subtraction (scores are bounded ~ +-1 for this
    problem's scaling) -> single ACT Exp pass PSUM->SBUF, FD=2048.
  - second matmul consumes exp(ST) directly as the moving operand with
    stationary [v | 1] (128, 33): row 32 accumulates the softmax denominator
    for free. Two heads share a PSUM bank via column-tiling (0 / 64).
  - normalize: gather denominators, reciprocal_approx_fast, gpsimd
    partition_broadcast, tensor_mul into SBUF.
  - final projection Y = Wout @ OT accumulated over the two head-quads.
"""

import numpy as np

B, M, N_CTX, C, H, W = 1, 4, 3, 64, 32, 32
HEADS, D = 8, 32
HWTOK = H * W          # 1024 tokens per frame
IB = 512               # q tokens per core
J = N_CTX * HWTOK      # 3072 kv tokens
NT = J // 128          # 24 j-tiles
GSTRIDE = 33 * HEADS   # 264: aug stride per j-tile in vts
SCALE = float(D) ** -0.5

_CACHE = {}


def _build_nc():
    import concourse.tile as tile
    from concourse import bacc, mybir

    F32 = mybir.dt.float32
    ACT_EXP = mybir.ActivationFunctionType.Exp

    nc = bacc.Bacc(
        "TRN2",
        target_bir_lowering=False,
        debug=False,
        enable_asserts=True,
        num_devices=8,
    )

    x1_d = nc.dram_tensor("x1c", (C, IB), F32, kind="ExternalInput").ap()
    x2_d = nc.dram_tensor("x2c", (C, J), F32, kind="ExternalInput").ap()
    wq_d = nc.dram_tensor("wqT", (C, 256), F32, kind="ExternalInput").ap()
    wk_d = nc.dram_tensor("wkT", (C, 256), F32, kind="ExternalInput").ap()
    wv_d = nc.dram_tensor("wvT", (C, 256), F32, kind="ExternalInput").ap()
    wo_d = nc.dram_tensor("woT", (128, 128), F32, kind="ExternalInput").ap()
    y_d = nc.dram_tensor("y", (C, IB), F32, kind="ExternalOutput").ap()

    with tile.TileContext(nc) as tc:
        from contextlib import ExitStack

        with ExitStack() as ctx:
            const = ctx.enter_context(tc.tile_pool(name="const", bufs=1))

            # ---- inputs to SBUF
            x1s = const.tile([C, IB], F32)
            nc.sync.dma_start(x1s[:], x1_d[:])
            x2s = const.tile([C, J], F32)
            nc.sync.dma_start(x2s[:, 0:1536], x2_d[:, 0:1536])
            nc.sync.dma_start(x2s[:, 1536:3072], x2_d[:, 1536:3072])
            wqs = const.tile([C, 256], F32)
            nc.sync.dma_start(wqs[:], wq_d[:])
            wks = const.tile([C, 256], F32)
            nc.sync.dma_start(wks[:], wk_d[:])
            wvs = const.tile([C, 256], F32)
            nc.sync.dma_start(wvs[:], wv_d[:])
            wos = const.tile([128, 128], F32)
            nc.sync.dma_start(wos[:], wo_d[:])

            # ---- persistent SBUF tensors
            qts = const.tile([128, 1024], F32)        # quad q at cols [512q:512q+512]
            kts = [
                const.tile([128, J], F32, name=f"kt{q}", tag=f"kt{q}")
                for q in range(2)
            ]
            vts = const.tile([128, NT * GSTRIDE], F32)  # [v | 1] aug, (j, head*33)
            ots_sb = [
                const.tile([128, IB], F32, name=f"osb{q}", tag=f"osb{q}")
                for q in range(2)
            ]
            ys = const.tile([C, IB], F32)

            # ---- projections
            with tc.tile_pool(name="proj_ps", bufs=3, space="PSUM") as ppool:
                # ones columns of vts (col 32 of each 33-wide head block)
                ones_v = vts[:].rearrange("p (t h x) -> p t h x", t=NT, x=33)[
                    :, :, :, 32:33
                ]
                nc.vector.memset(ones_v, 1.0)

                # QT = WqT.T @ x1  -> (256, 512), quads side by side
                qp = ppool.tile([128, 1024], F32, tag="proj", name="qp")
                for q in range(2):
                    nc.tensor.matmul(
                        qp[:, 512 * q : 512 * (q + 1)],
                        wqs[:, 128 * q : 128 * (q + 1)],
                        x1s[:],
                        start=True,
                        stop=True,
                    )
                nc.vector.tensor_copy(qts[:], qp[:])

                # KT = WkT.T @ x2 -> 2 quad tiles (128, 3072)
                for q in range(2):
                    for jb in range(3):
                        kp = ppool.tile([128, 1024], F32, tag="proj", name="kp")
                        for s in range(2):
                            nc.tensor.matmul(
                                kp[:, 512 * s : 512 * (s + 1)],
                                wks[:, 128 * q : 128 * (q + 1)],
                                x2s[:, 1024 * jb + 512 * s : 1024 * jb + 512 * (s + 1)],
                                start=True,
                                stop=True,
                            )
                        nc.scalar.copy(kts[q][:, 1024 * jb : 1024 * (jb + 1)], kp[:])

                # VT = x2.T @ WvT -> (3072, 256) into aug layout (skip ones col)
                for tp in range(6):
                    vp = ppool.tile([128, 1024], F32, tag="proj", name="vp")
                    for s in range(4):
                        t = 4 * tp + s
                        nc.tensor.matmul(
                            vp[:, 256 * s : 256 * (s + 1)],
                            x2s[:, 128 * t : 128 * (t + 1)],
                            wvs[:],
                            start=True,
                            stop=True,
                        )
                    dst = vts[
                        :, 4 * GSTRIDE * tp : 4 * GSTRIDE * (tp + 1)
                    ].rearrange("p (t h x) -> p t h x", t=4, x=33)[:, :, :, 0:32]
                    src = vp[:].rearrange("p (t h x) -> p t h x", t=4, x=32)
                    nc.vector.tensor_copy(dst, src)

            # ---- attention main loop
            with ExitStack() as mctx:
                otp = mctx.enter_context(
                    tc.tile_pool(name="ot_ps", bufs=1, space="PSUM")
                )
                simp = mctx.enter_context(
                    tc.tile_pool(name="sim_ps", bufs=1, space="PSUM")
                )
                ptsp = mctx.enter_context(tc.tile_pool(name="pts_sb", bufs=2))
                epi = mctx.enter_context(tc.tile_pool(name="epi_sb", bufs=1))

                ots = [
                    otp.tile([128, IB], F32, name=f"otb{k}", tag=f"otb{k}")
                    for k in range(4)
                ]

                for q in range(2):
                    for t in range(NT):
                        st = simp.tile([128, 2048], F32, tag="st", name="st")
                        for r in range(4):
                            nc.tensor.matmul(
                                st[:, 512 * r : 512 * (r + 1)],
                                kts[q][32 * r : 32 * (r + 1), 128 * t : 128 * (t + 1)],
                                qts[32 * r : 32 * (r + 1), 512 * q : 512 * (q + 1)],
                                start=True,
                                stop=True,
                                tile_position=(32 * r, 0),
                            )
                        pt = ptsp.tile([128, 2048], F32, tag="pt", name="pt")
                        nc.scalar.activation(pt[:], st[:], ACT_EXP, scale=SCALE)
                        for r in range(4):
                            h = 4 * q + r
                            k = h // 2
                            bp = 64 * (r % 2)
                            nc.tensor.matmul(
                                ots[k][bp : bp + 33, :],
                                vts[:, GSTRIDE * t + 33 * h : GSTRIDE * t + 33 * (h + 1)],
                                pt[:, 512 * r : 512 * (r + 1)],
                                start=(t == 0),
                                stop=(t == NT - 1),
                                tile_position=(0, bp),
                                skip_group_check=True,
                            )

                    # epilogue for quad q (overlaps the next quad's main loop)
                    for r in range(4):
                        h = 4 * q + r
                        k = h // 2
                        bp = 64 * (r % 2)
                        den = epi.tile([1, IB], F32, tag=f"den{h}", name=f"den{h}")
                        nc.vector.tensor_copy(den[:], ots[k][bp + 32 : bp + 33, :])
                        rec = epi.tile([1, IB], F32, tag=f"rec{h}", name=f"rec{h}")
                        nc.vector.reciprocal_approx_fast(rec[:], den[:])
                        bca = epi.tile([32, IB], F32, tag=f"bca{h}", name=f"bca{h}")
                        nc.gpsimd.partition_broadcast(bca[:], rec[:], channels=32)
                        nc.vector.tensor_mul(
                            ots_sb[q][32 * r : 32 * (r + 1), :],
                            ots[k][bp : bp + 32, :],
                            bca[:],
                        )

            # ---- final projection y = WoutT.T @ OT (accumulate over quads)
            with tc.tile_pool(name="tail_ps", bufs=1, space="PSUM") as tailp:
                yp = tailp.tile([C, IB], F32)
                for q in range(2):
                    nc.tensor.matmul(
                        yp[:],
                        wos[:, 64 * q : 64 * (q + 1)],
                        ots_sb[q][:],
                        start=(q == 0),
                        stop=(q == 1),
                    )
                nc.vector.tensor_copy(ys[:], yp[:])
            nc.sync.dma_start(y_d[:], ys[:])

    nc.compile()
    return nc


def _prep_core_inputs(x1, x2, Wq, Wkv, Wout):
    x1 = np.asarray(x1, dtype=np.float32)
    x2 = np.asarray(x2, dtype=np.float32)
    Wq = np.asarray(Wq, dtype=np.float32)
    Wkv = np.asarray(Wkv, dtype=np.float32)
    Wout = np.asarray(Wout, dtype=np.float32)

    wqT = np.ascontiguousarray(Wq.T)                      # (64, 256)
    wkT = np.ascontiguousarray(Wkv[:256].T)               # (64, 256)
    wvT = np.ascontiguousarray(Wkv[256:].T)               # (64, 256)
    # WoutT (256, 64) packed as (128, 128): chunk q at cols [64q:64q+64]
    woT = np.ascontiguousarray(
        Wout.T.reshape(2, 128, 64).transpose(1, 0, 2).reshape(128, 128)
    )

    in_maps = []
    for f in range(M):
        x1f = x1[0, f].reshape(C, HWTOK)                          # (64, 1024)
        x2f = np.ascontiguousarray(
            x2[0, f].transpose(1, 0, 2, 3).reshape(C, J)          # (64, 3072)
        )
        for half in range(2):
            in_maps.append(
                {
                    "x1c": np.ascontiguousarray(x1f[:, IB * half : IB * (half + 1)]),
                    "x2c": x2f,
                    "wqT": wqT,
                    "wkT": wkT,
                    "wvT": wvT,
                    "woT": woT,
                }
            )
    return in_maps


def kernel(x1, x2, Wq, Wkv, Wout):
    from concourse.bass_utils import run_bass_kernel_spmd

    if "nc" not in _CACHE:
        _CACHE["nc"] = _build_nc()
    nc = _CACHE["nc"]

    in_maps = _prep_core_inputs(x1, x2, Wq, Wkv, Wout)
    res = run_bass_kernel_spmd(nc, in_maps, core_ids=list(range(8)))

    out = np.empty((B, M, C, H, W), dtype=np.float32)
    for f in range(M):
        yf = np.empty((C, HWTOK), dtype=np.float32)
        for half in range(2):
            yf[:, IB * half : IB * (half + 1)] = res.results[2 * f + half]["y"]
        out[0, f] = yf.reshape(C, H, W)
    return out


# revision 5
# speedup vs baseline: 1.7732x; 1.7732x over previous
"""Trainium2 Bass kernel for nn_CrossAttention (b,m,c,H,W cross-attention).

Problem (hardcoded shapes): b=1, m=4, n=3, c=64, H=W=32, heads=8, dim_head=32.

  q  = Wq  @ x1   per frame        (256, 1024)
  kv = Wkv @ x2   per frame        (512, 3072)
  per (frame, head): attn softmax((q k^T)/sqrt(d)) @ v,  d=32
  y  = Wout @ out  per frame       (64, 1024)

Sharding: 8 cores = 4 frames x 2 q-token halves. Each core gets all 8 heads,
512 q tokens, the full 3072 kv tokens of its frame. No cross-core comms;
outputs concatenate.

Per-core kernel layout strategy:
  - QT (256, 512) / KT (256, 3072) channel-major from 1x1-conv projections,
    heads at 32-partition offsets (quad tiles of 4 heads x 32 dims = 128).
  - scores computed TRANSPOSED: ST[j, i] = sum_d k[j,d] q[i,d] via PE
    row-tiling (4 heads concurrently, K=32 each at tile_position (32r, 0)).
  - softmax without max-subtraction (scores are bounded ~ +-1 for this
    problem's scaling) -> single ACT Exp pass PSUM->SBUF, FD=2048.
  - second matmul consumes exp(ST) directly as the moving operand with
    stationary [v | 1] (128, 33): row 32 accumulates the softmax denominator
    for free. Two heads share a PSUM bank via column-tiling (0 / 64).
  - normalize: gather denominators, reciprocal_approx_fast, gpsimd
    partition_broadcast, tensor_mul into SBUF.
  - final projection Y = Wout @ OT accumulated over the two head-quads.
"""

import numpy as np

B, M, N_CTX, C, H, W = 1, 4, 3, 64, 32, 32
HEADS, D = 8, 32
HWTOK = H * W          # 1024 tokens per frame
IB = 512               # q tokens per core
J = N_CTX * HWTOK      # 3072 kv tokens
NT = J // 128          # 24 j-tiles
GSTRIDE = 33 * HEADS   # 264: aug stride per j-tile in vts
SCALE = float(D) ** -0.5

_CACHE = {}


def _build_nc():
    import concourse.tile as tile
    from concourse import bacc, mybir

    F32 = mybir.dt.float32
    BF16 = mybir.dt.bfloat16
    ACT_EXP = mybir.ActivationFunctionType.Exp

    nc = bacc.Bacc(
        "TRN2",
        target_bir_lowering=False,
        debug=False,
        enable_asserts=True,
        num_devices=8,
    )

    x1_d = nc.dram_tensor("x1c", (C, IB), F32, kind="ExternalInput").ap()
    x2_d = nc.dram_tensor("x2c", (C, J), F32, kind="ExternalInput").ap()
    wq_d = nc.dram_tensor("wqT", (C, 256), F32, kind="ExternalInput").ap()
    wk_d = nc.dram_tensor("wkT", (C, 256), F32, kind="ExternalInput").ap()
    wv_d = nc.dram_tensor("wvT", (C, 256), F32, kind="ExternalInput").ap()
    wo_d = nc.dram_tensor("woT", (128, 128), F32, kind="ExternalInput").ap()
    y_d = nc.dram_tensor("y", (C, IB), F32, kind="ExternalOutput").ap()

    with tile.TileContext(nc) as tc:
        from contextlib import ExitStack

        with ExitStack() as ctx:
            const = ctx.enter_context(tc.tile_pool(name="const", bufs=1))

            # ---- inputs to SBUF (fp32 staging), convert to bf16 for the PE
            # (fp32 matmuls run fp32_mode=LOW_HIGH = 2x streaming passes, so
            # every PE operand except the final projection is bf16)
            x1f = const.tile([C, IB], F32)
            nc.sync.dma_start(x1f[:], x1_d[:])
            x2f = const.tile([C, J], F32)
            nc.sync.dma_start(x2f[:, 0:1536], x2_d[:, 0:1536])
            nc.sync.dma_start(x2f[:, 1536:3072], x2_d[:, 1536:3072])
            wqf = const.tile([C, 256], F32)
            nc.sync.dma_start(wqf[:], wq_d[:])
            wkf = const.tile([C, 256], F32)
            nc.sync.dma_start(wkf[:], wk_d[:])
            wvf = const.tile([C, 256], F32)
            nc.sync.dma_start(wvf[:], wv_d[:])
            wos = const.tile([128, 128], F32)
            nc.sync.dma_start(wos[:], wo_d[:])

            x1s = const.tile([C, IB], BF16)
            nc.vector.tensor_copy(x1s[:], x1f[:])
            x2s = const.tile([C, J], BF16)
            nc.vector.tensor_copy(x2s[:, 0:1536], x2f[:, 0:1536])
            nc.vector.tensor_copy(x2s[:, 1536:3072], x2f[:, 1536:3072])
            wqs = const.tile([C, 256], BF16)
            nc.vector.tensor_copy(wqs[:], wqf[:])
            wks = const.tile([C, 256], BF16)
            nc.vector.tensor_copy(wks[:], wkf[:])
            wvs = const.tile([C, 256], BF16)
            nc.vector.tensor_copy(wvs[:], wvf[:])

            # ---- persistent SBUF tensors (attention operands in bf16:
            # fp32 matmuls run fp32_mode=LOW_HIGH = 2x streaming passes)
            qts = const.tile([128, 1024], BF16)       # quad q at cols [512q:512q+512]
            kts = [
                const.tile([128, J], BF16, name=f"kt{q}", tag=f"kt{q}")
                for q in range(2)
            ]
            vts = const.tile([128, NT * GSTRIDE], BF16)  # [v | 1] aug, (j, head*33)
            ots_sb = [
                const.tile([128, IB], F32, name=f"osb{q}", tag=f"osb{q}")
                for q in range(2)
            ]
            ys = const.tile([C, IB], F32)

            # ---- projections
            with tc.tile_pool(name="proj_ps", bufs=3, space="PSUM") as ppool:
                # ones columns of vts (col 32 of each 33-wide head block)
                ones_v = vts[:].rearrange("p (t h x) -> p t h x", t=NT, x=33)[
                    :, :, :, 32:33
                ]
                nc.vector.memset(ones_v, 1.0)

                # QT = WqT.T @ x1  -> (256, 512), quads side by side
                qp = ppool.tile([128, 1024], F32, tag="proj", name="qp")
                for q in range(2):
                    nc.tensor.matmul(
                        qp[:, 512 * q : 512 * (q + 1)],
                        wqs[:, 128 * q : 128 * (q + 1)],
                        x1s[:],
                        start=True,
                        stop=True,
                    )
                nc.vector.tensor_copy(qts[:], qp[:])

                # KT = WkT.T @ x2 -> 2 quad tiles (128, 3072)
                for q in range(2):
                    for jb in range(3):
                        kp = ppool.tile([128, 1024], F32, tag="proj", name="kp")
                        for s in range(2):
                            nc.tensor.matmul(
                                kp[:, 512 * s : 512 * (s + 1)],
                                wks[:, 128 * q : 128 * (q + 1)],
                                x2s[:, 1024 * jb + 512 * s : 1024 * jb + 512 * (s + 1)],
                                start=True,
                                stop=True,
                            )
                        nc.scalar.copy(kts[q][:, 1024 * jb : 1024 * (jb + 1)], kp[:])

                # VT = x2.T @ WvT -> (3072, 256) into aug layout (skip ones col)
                for tp in range(6):
                    vp = ppool.tile([128, 1024], F32, tag="proj", name="vp")
                    for s in range(4):
                        t = 4 * tp + s
                        nc.tensor.matmul(
                            vp[:, 256 * s : 256 * (s + 1)],
                            x2s[:, 128 * t : 128 * (t + 1)],
                            wvs[:],
                            start=True,
                            stop=True,
                        )
                    dst = vts[
                        :, 4 * GSTRIDE * tp : 4 * GSTRIDE * (tp + 1)
                    ].rearrange("p (t h x) -> p t h x", t=4, x=33)[:, :, :, 0:32]
                    src = vp[:].rearrange("p (t h x) -> p t h x", t=4, x=32)
                    nc.vector.tensor_copy(dst, src)

            # ---- attention main loop
            with ExitStack() as mctx:
                otp = mctx.enter_context(
                    tc.tile_pool(name="ot_ps", bufs=1, space="PSUM")
                )
                simp = mctx.enter_context(
                    tc.tile_pool(name="sim_ps", bufs=1, space="PSUM")
                )
                ptsp = mctx.enter_context(tc.tile_pool(name="pts_sb", bufs=2))
                epi = mctx.enter_context(tc.tile_pool(name="epi_sb", bufs=1))

                ots = [
                    otp.tile([128, IB], F32, name=f"otb{k}", tag=f"otb{k}")
                    for k in range(4)
                ]

                for q in range(2):
                    for t in range(NT):
                        st = simp.tile([128, 2048], F32, tag="st", name="st")
                        for r in range(4):
                            nc.tensor.matmul(
                                st[:, 512 * r : 512 * (r + 1)],
                                kts[q][32 * r : 32 * (r + 1), 128 * t : 128 * (t + 1)],
                                qts[32 * r : 32 * (r + 1), 512 * q : 512 * (q + 1)],
                                start=True,
                                stop=True,
                                tile_position=(32 * r, 0),
                            )
                        pt = ptsp.tile([128, 2048], BF16, tag="pt", name="pt")
                        nc.scalar.activation(pt[:], st[:], ACT_EXP, scale=SCALE)
                        for r in range(4):
                            h = 4 * q + r
                            k = h // 2
                            bp = 64 * (r % 2)
                            nc.tensor.matmul(
                                ots[k][bp : bp + 33, :],
                                vts[:, GSTRIDE * t + 33 * h : GSTRIDE * t + 33 * (h + 1)],
                                pt[:, 512 * r : 512 * (r + 1)],
                                start=(t == 0),
                                stop=(t == NT - 1),
                                tile_position=(0, bp),
                                skip_group_check=True,
                            )

                    # epilogue for quad q (overlaps the next quad's main loop)
                    for r in range(4):
                        h = 4 * q + r
                        k = h // 2
                        bp = 64 * (r % 2)
                        den = epi.tile([1, IB], F32, tag=f"den{h}", name=f"den{h}")
                        nc.vector.tensor_copy(den[:], ots[k][bp + 32 : bp + 33, :])
                        rec = epi.tile([1, IB], F32, tag=f"rec{h}", name=f"rec{h}")
                        nc.vector.reciprocal_approx_fast(rec[:], den[:])
                        bca = epi.tile([32, IB], F32, tag=f"bca{h}", name=f"bca{h}")
                        nc.gpsimd.partition_broadcast(bca[:], rec[:], channels=32)
                        nc.vector.tensor_mul(
                            ots_sb[q][32 * r : 32 * (r + 1), :],
                            ots[k][bp : bp + 32, :],
                            bca[:],
                        )

            # ---- final projection y = WoutT.T @ OT (accumulate over quads)
            with tc.tile_pool(name="tail_ps", bufs=1, space="PSUM") as tailp:
                yp = tailp.tile([C, IB], F32)
                for q in range(2):
                    nc.tensor.matmul(
                        yp[:],
                        wos[:, 64 * q : 64 * (q + 1)],
                        ots_sb[q][:],
                        start=(q == 0),
                        stop=(q == 1),
                    )
                nc.vector.tensor_copy(ys[:], yp[:])
            nc.sync.dma_start(y_d[:], ys[:])

    nc.compile()
    return nc


def _prep_core_inputs(x1, x2, Wq, Wkv, Wout):
    x1 = np.asarray(x1, dtype=np.float32)
    x2 = np.asarray(x2, dtype=np.float32)
    Wq = np.asarray(Wq, dtype=np.float32)
    Wkv = np.asarray(Wkv, dtype=np.float32)
    Wout = np.asarray(Wout, dtype=np.float32)

    wqT = np.ascontiguousarray(Wq.T)                      # (64, 256)
    wkT = np.ascontiguousarray(Wkv[:256].T)               # (64, 256)
    wvT = np.ascontiguousarray(Wkv[256:].T)               # (64, 256)
    # WoutT (256, 64) packed as (128, 128): chunk q at cols [64q:64q+64]
    woT = np.ascontiguousarray(
        Wout.T.reshape(2, 128, 64).transpose(1, 0, 2).reshape(128, 128)
    )

    in_maps = []
    for f in range(M):
        x1f = x1[0, f].reshape(C, HWTOK)                          # (64, 1024)
        x2f = np.ascontiguousarray(
            x2[0, f].transpose(1, 0, 2, 3).reshape(C, J)          # (64, 3072)
        )
        for half in range(2):
            in_maps.append(
                {
                    "x1c": np.ascontiguousarray(x1f[:, IB * half : IB * (half + 1)]),
                    "x2c": x2f,
                    "wqT": wqT,
                    "wkT": wkT,
                    "wvT": wvT,
                    "woT": woT,
                }
            )
    return in_maps


def kernel(x1, x2, Wq, Wkv, Wout):
    from concourse.bass_utils import run_bass_kernel_spmd

    if "nc" not in _CACHE:
        _CACHE["nc"] = _build_nc()
    nc = _CACHE["nc"]

    in_maps = _prep_core_inputs(x1, x2, Wq, Wkv, Wout)
    res = run_bass_kernel_spmd(nc, in_maps, core_ids=list(range(8)))

    out = np.empty((B, M, C, H, W), dtype=np.float32)
    for f in range(M):
        yf = np.empty((C, HWTOK), dtype=np.float32)
        for half in range(2):
            yf[:, IB * half : IB * (half + 1)] = res.results[2 * f + half]["y"]
        out[0, f] = yf.reshape(C, H, W)
    return out


# revision 6
# speedup vs baseline: 2.4606x; 1.3876x over previous
"""Trainium2 Bass kernel for nn_CrossAttention (b,m,c,H,W cross-attention).

Problem (hardcoded shapes): b=1, m=4, n=3, c=64, H=W=32, heads=8, dim_head=32.

  q  = Wq  @ x1   per frame        (256, 1024)
  kv = Wkv @ x2   per frame        (512, 3072)
  per (frame, head): attn softmax((q k^T)/sqrt(d)) @ v,  d=32
  y  = Wout @ out  per frame       (64, 1024)

Sharding: 8 cores = 4 frames x 2 q-token halves. Each core gets all 8 heads,
512 q tokens, the full 3072 kv tokens of its frame. No cross-core comms;
outputs concatenate.

Per-core kernel layout strategy:
  - QT (256, 512) / KT (256, 3072) channel-major from 1x1-conv projections,
    heads at 32-partition offsets (quad tiles of 4 heads x 32 dims = 128).
  - scores computed TRANSPOSED: ST[j, i] = sum_d k[j,d] q[i,d] via PE
    row-tiling (4 heads concurrently, K=32 each at tile_position (32r, 0)).
  - softmax without max-subtraction (scores are bounded ~ +-1 for this
    problem's scaling) -> single ACT Exp pass PSUM->SBUF, FD=2048.
  - second matmul consumes exp(ST) directly as the moving operand with
    stationary [v | 1] (128, 33): row 32 accumulates the softmax denominator
    for free. Two heads share a PSUM bank via column-tiling (0 / 64).
  - normalize: gather denominators, reciprocal_approx_fast, gpsimd
    partition_broadcast, tensor_mul into SBUF.
  - final projection Y = Wout @ OT accumulated over the two head-quads.
"""

import numpy as np

B, M, N_CTX, C, H, W = 1, 4, 3, 64, 32, 32
HEADS, D = 8, 32
HWTOK = H * W          # 1024 tokens per frame
IB = 512               # q tokens per core
J = N_CTX * HWTOK      # 3072 kv tokens
NT = J // 128          # 24 j-tiles
GSTRIDE = 33 * HEADS   # 264: aug stride per j-tile in vts
SCALE = float(D) ** -0.5

_CACHE = {}


def _build_nc():
    import concourse.tile as tile
    from concourse import bacc, mybir

    F32 = mybir.dt.float32
    BF16 = mybir.dt.bfloat16
    ACT_EXP = mybir.ActivationFunctionType.Exp

    nc = bacc.Bacc(
        "TRN2",
        target_bir_lowering=False,
        debug=False,
        enable_asserts=True,
        num_devices=8,
    )

    x1_d = nc.dram_tensor("x1c", (C, IB), F32, kind="ExternalInput").ap()
    x2_d = nc.dram_tensor("x2c", (C, J), F32, kind="ExternalInput").ap()
    wq_d = nc.dram_tensor("wqT", (C, 256), F32, kind="ExternalInput").ap()
    wk_d = nc.dram_tensor("wkT", (C, 256), F32, kind="ExternalInput").ap()
    wv_d = nc.dram_tensor("wvT", (C, 256), F32, kind="ExternalInput").ap()
    wo_d = nc.dram_tensor("woT", (128, 128), F32, kind="ExternalInput").ap()
    y_d = nc.dram_tensor("y", (C, IB), F32, kind="ExternalOutput").ap()

    with tile.TileContext(nc) as tc:
        from contextlib import ExitStack

        with ExitStack() as ctx:
            const = ctx.enter_context(tc.tile_pool(name="const", bufs=1))

            # ---- inputs to SBUF (fp32 staging), convert to bf16 for the PE
            # (fp32 matmuls run fp32_mode=LOW_HIGH = 2x streaming passes, so
            # every PE operand except the final projection is bf16)
            x1f = const.tile([C, IB], F32)
            nc.sync.dma_start(x1f[:], x1_d[:])
            x2f = const.tile([C, J], F32)
            nc.sync.dma_start(x2f[:, 0:1536], x2_d[:, 0:1536])
            nc.sync.dma_start(x2f[:, 1536:3072], x2_d[:, 1536:3072])
            wqf = const.tile([C, 256], F32)
            nc.sync.dma_start(wqf[:], wq_d[:])
            wkf = const.tile([C, 256], F32)
            nc.sync.dma_start(wkf[:], wk_d[:])
            wvf = const.tile([C, 256], F32)
            nc.sync.dma_start(wvf[:], wv_d[:])
            wos = const.tile([128, 128], F32)
            nc.sync.dma_start(wos[:], wo_d[:])

            x1s = const.tile([C, IB], BF16)
            nc.vector.tensor_copy(x1s[:], x1f[:])
            x2s = const.tile([C, J], BF16)
            nc.vector.tensor_copy(x2s[:, 0:1536], x2f[:, 0:1536])
            nc.vector.tensor_copy(x2s[:, 1536:3072], x2f[:, 1536:3072])
            wqs = const.tile([C, 256], BF16)
            nc.vector.tensor_copy(wqs[:], wqf[:])
            wks = const.tile([C, 256], BF16)
            nc.vector.tensor_copy(wks[:], wkf[:])
            wvs = const.tile([C, 256], BF16)
            nc.vector.tensor_copy(wvs[:], wvf[:])

            # ---- persistent SBUF tensors (attention operands in bf16:
            # fp32 matmuls run fp32_mode=LOW_HIGH = 2x streaming passes)
            qts = const.tile([128, 1024], BF16)       # quad q at cols [512q:512q+512]
            kts = [
                const.tile([128, J], BF16, name=f"kt{q}", tag=f"kt{q}")
                for q in range(2)
            ]
            vts = const.tile([128, NT * GSTRIDE], BF16)  # [v | 1] aug, (j, head*33)
            ots_sb = [
                const.tile([128, IB], F32, name=f"osb{q}", tag=f"osb{q}")
                for q in range(2)
            ]
            ys = const.tile([C, IB], F32)

            # ---- projections (resident 2-bank PSUM pool; pieces are
            # interleaved with the early main loop so the PE stays dense and
            # the exp stream starts ~5us in instead of ~18us)
            ppool = ctx.enter_context(
                tc.tile_pool(name="proj_ps", bufs=1, space="PSUM")
            )

            # ones columns of vts (col 32 of each 33-wide head block)
            ones_v = vts[:].rearrange("p (t h x) -> p t h x", t=NT, x=33)[
                :, :, :, 32:33
            ]
            nc.vector.memset(ones_v, 1.0)

            def emit_qt():
                qp = ppool.tile([128, 1024], F32, tag="proj", name="qp")
                for q in range(2):
                    nc.tensor.matmul(
                        qp[:, 512 * q : 512 * (q + 1)],
                        wqs[:, 128 * q : 128 * (q + 1)],
                        x1s[:],
                        start=True,
                        stop=True,
                    )
                nc.vector.tensor_copy(qts[:], qp[:])

            def emit_kt(q, jb):
                kp = ppool.tile([128, 1024], F32, tag="proj", name="kp")
                for s in range(2):
                    nc.tensor.matmul(
                        kp[:, 512 * s : 512 * (s + 1)],
                        wks[:, 128 * q : 128 * (q + 1)],
                        x2s[:, 1024 * jb + 512 * s : 1024 * jb + 512 * (s + 1)],
                        start=True,
                        stop=True,
                    )
                nc.vector.tensor_copy(kts[q][:, 1024 * jb : 1024 * (jb + 1)], kp[:])

            def emit_vt(tp):
                vp = ppool.tile([128, 1024], F32, tag="proj", name="vp")
                for s in range(4):
                    t = 4 * tp + s
                    nc.tensor.matmul(
                        vp[:, 256 * s : 256 * (s + 1)],
                        x2s[:, 128 * t : 128 * (t + 1)],
                        wvs[:],
                        start=True,
                        stop=True,
                    )
                dst = vts[
                    :, 4 * GSTRIDE * tp : 4 * GSTRIDE * (tp + 1)
                ].rearrange("p (t h x) -> p t h x", t=4, x=33)[:, :, :, 0:32]
                src = vp[:].rearrange("p (t h x) -> p t h x", t=4, x=32)
                nc.vector.tensor_copy(dst, src)

            emit_qt()
            emit_kt(0, 0)
            emit_vt(0)
            # remaining pieces, emitted one per early group of pair 0
            # (deadlines: vt(i) by t=4i, kt(0,jb) by t=8jb, kt(1,*) by pair 2)
            pending = [
                lambda: emit_vt(1),
                lambda: emit_vt(2),
                lambda: emit_kt(0, 1),
                lambda: emit_vt(3),
                lambda: emit_kt(0, 2),
                lambda: emit_vt(4),
                lambda: emit_vt(5),
                lambda: emit_kt(1, 0),
                lambda: emit_kt(1, 1),
                lambda: emit_kt(1, 2),
            ]

            # ---- attention main loop: 2-head groups, double-buffered sim
            # PSUM (2 banks x 2 bufs) + 2 OT accumulator banks (pairs reuse
            # them) + 2 resident proj banks = 8 banks total
            with ExitStack() as mctx:
                otp = mctx.enter_context(
                    tc.tile_pool(name="ot_ps", bufs=1, space="PSUM")
                )
                simp = mctx.enter_context(
                    tc.tile_pool(name="sim_ps", bufs=2, space="PSUM")
                )
                ptsp = mctx.enter_context(tc.tile_pool(name="pts_sb", bufs=3))
                epi = mctx.enter_context(tc.tile_pool(name="epi_sb", bufs=1))

                for p in range(4):
                    q = p // 2
                    otb = otp.tile(
                        [128, IB], F32, tag=f"otb{p % 2}", name=f"otb{p}"
                    )
                    for t in range(NT):
                        if p == 0 and 1 <= t <= len(pending):
                            pending[t - 1]()
                        st = simp.tile([128, 1024], F32, tag="st", name="st")
                        for s in range(2):
                            h = 2 * p + s
                            rl = h % 4
                            nc.tensor.matmul(
                                st[:, 512 * s : 512 * (s + 1)],
                                kts[q][
                                    32 * rl : 32 * (rl + 1),
                                    128 * t : 128 * (t + 1),
                                ],
                                qts[32 * rl : 32 * (rl + 1), 512 * q : 512 * (q + 1)],
                                start=True,
                                stop=True,
                                tile_position=(32 * rl, 0),
                            )
                        pt = ptsp.tile([128, 1024], BF16, tag="pt", name="pt")
                        nc.scalar.activation(pt[:], st[:], ACT_EXP, scale=SCALE)
                        for s in range(2):
                            h = 2 * p + s
                            bp = 64 * s
                            nc.tensor.matmul(
                                otb[bp : bp + 33, :],
                                vts[:, GSTRIDE * t + 33 * h : GSTRIDE * t + 33 * (h + 1)],
                                pt[:, 512 * s : 512 * (s + 1)],
                                start=(t == 0),
                                stop=(t == NT - 1),
                                tile_position=(0, bp),
                                skip_group_check=True,
                            )

                    # epilogue for pair p (overlaps the next pair's main loop)
                    for s in range(2):
                        h = 2 * p + s
                        rl = h % 4
                        bp = 64 * s
                        den = epi.tile([1, IB], F32, tag=f"den{h}", name=f"den{h}")
                        nc.vector.tensor_copy(den[:], otb[bp + 32 : bp + 33, :])
                        rec = epi.tile([1, IB], F32, tag=f"rec{h}", name=f"rec{h}")
                        nc.vector.reciprocal_approx_fast(rec[:], den[:])
                        bca = epi.tile([32, IB], F32, tag=f"bca{h}", name=f"bca{h}")
                        nc.gpsimd.partition_broadcast(bca[:], rec[:], channels=32)
                        nc.vector.tensor_mul(
                            ots_sb[q][32 * rl : 32 * (rl + 1), :],
                            otb[bp : bp + 32, :],
                            bca[:],
                        )

            # ---- final projection y = WoutT.T @ OT (accumulate over quads)
            with tc.tile_pool(name="tail_ps", bufs=1, space="PSUM") as tailp:
                yp = tailp.tile([C, IB], F32)
                for q in range(2):
                    nc.tensor.matmul(
                        yp[:],
                        wos[:, 64 * q : 64 * (q + 1)],
                        ots_sb[q][:],
                        start=(q == 0),
                        stop=(q == 1),
                    )
                nc.vector.tensor_copy(ys[:], yp[:])
            nc.sync.dma_start(y_d[:], ys[:])

    nc.compile()
    return nc


def _prep_core_inputs(x1, x2, Wq, Wkv, Wout):
    x1 = np.asarray(x1, dtype=np.float32)
    x2 = np.asarray(x2, dtype=np.float32)
    Wq = np.asarray(Wq, dtype=np.float32)
    Wkv = np.asarray(Wkv, dtype=np.float32)
    Wout = np.asarray(Wout, dtype=np.float32)

    wqT = np.ascontiguousarray(Wq.T)                      # (64, 256)
    wkT = np.ascontiguousarray(Wkv[:256].T)               # (64, 256)
    wvT = np.ascontiguousarray(Wkv[256:].T)               # (64, 256)
    # WoutT (256, 64) packed as (128, 128): chunk q at cols [64q:64q+64]
    woT = np.ascontiguousarray(
        Wout.T.reshape(2, 128, 64).transpose(1, 0, 2).reshape(128, 128)
    )

    in_maps = []
    for f in range(M):
        x1f = x1[0, f].reshape(C, HWTOK)                          # (64, 1024)
        x2f = np.ascontiguousarray(
            x2[0, f].transpose(1, 0, 2, 3).reshape(C, J)          # (64, 3072)
        )
        for half in range(2):
            in_maps.append(
                {
                    "x1c": np.ascontiguousarray(x1f[:, IB * half : IB * (half + 1)]),
                    "x2c": x2f,
                    "wqT": wqT,
                    "wkT": wkT,
                    "wvT": wvT,
                    "woT": woT,
                }
            )
    return in_maps


def kernel(x1, x2, Wq, Wkv, Wout):
    from concourse.bass_utils import run_bass_kernel_spmd

    if "nc" not in _CACHE:
        _CACHE["nc"] = _build_nc()
    nc = _CACHE["nc"]

    in_maps = _prep_core_inputs(x1, x2, Wq, Wkv, Wout)
    res = run_bass_kernel_spmd(nc, in_maps, core_ids=list(range(8)))

    out = np.empty((B, M, C, H, W), dtype=np.float32)
    for f in range(M):
        yf = np.empty((C, HWTOK), dtype=np.float32)
        for half in range(2):
            yf[:, IB * half : IB * (half + 1)] = res.results[2 * f + half]["y"]
        out[0, f] = yf.reshape(C, H, W)
    return out


# revision 10
# speedup vs baseline: 2.5385x; 1.0317x over previous
"""Trainium2 Bass kernel for nn_CrossAttention (b,m,c,H,W cross-attention).

Problem (hardcoded shapes): b=1, m=4, n=3, c=64, H=W=32, heads=8, dim_head=32.

  q  = Wq  @ x1   per frame        (256, 1024)
  kv = Wkv @ x2   per frame        (512, 3072)
  per (frame, head): attn softmax((q k^T)/sqrt(d)) @ v,  d=32
  y  = Wout @ out  per frame       (64, 1024)

Sharding: 8 cores = 4 frames x 2 q-token halves. Each core gets all 8 heads,
512 q tokens, the full 3072 kv tokens of its frame. No cross-core comms;
outputs concatenate.

Per-core kernel layout strategy:
  - QT (256, 512) / KT (256, 3072) channel-major from 1x1-conv projections,
    heads at 32-partition offsets (quad tiles of 4 heads x 32 dims = 128).
  - scores computed TRANSPOSED: ST[j, i] = sum_d k[j,d] q[i,d] via PE
    row-tiling (4 heads concurrently, K=32 each at tile_position (32r, 0)).
  - softmax without max-subtraction (scores are bounded ~ +-1 for this
    problem's scaling) -> single ACT Exp pass PSUM->SBUF, FD=2048.
  - second matmul consumes exp(ST) directly as the moving operand with
    stationary [v | 1] (128, 33): row 32 accumulates the softmax denominator
    for free. Two heads share a PSUM bank via column-tiling (0 / 64).
  - normalize: gather denominators, reciprocal_approx_fast, gpsimd
    partition_broadcast, tensor_mul into SBUF.
  - final projection Y = Wout @ OT accumulated over the two head-quads.
"""

import numpy as np

B, M, N_CTX, C, H, W = 1, 4, 3, 64, 32, 32
HEADS, D = 8, 32
HWTOK = H * W          # 1024 tokens per frame
IB = 512               # q tokens per core
J = N_CTX * HWTOK      # 3072 kv tokens
NT = J // 128          # 24 j-tiles
GSTRIDE = 33 * HEADS   # 264: aug stride per j-tile in vts
SCALE = float(D) ** -0.5

_CACHE = {}


def _build_nc():
    import concourse.tile as tile
    from concourse import bacc, mybir

    F32 = mybir.dt.float32
    BF16 = mybir.dt.bfloat16
    ACT_EXP = mybir.ActivationFunctionType.Exp

    nc = bacc.Bacc(
        "TRN2",
        target_bir_lowering=False,
        debug=False,
        enable_asserts=True,
        num_devices=8,
    )

    x1_d = nc.dram_tensor("x1c", (C, IB), F32, kind="ExternalInput").ap()
    x2_d = nc.dram_tensor("x2c", (C, J), F32, kind="ExternalInput").ap()
    wq_d = nc.dram_tensor("wqT", (C, 256), F32, kind="ExternalInput").ap()
    wk_d = nc.dram_tensor("wkT", (C, 256), F32, kind="ExternalInput").ap()
    wv_d = nc.dram_tensor("wvT", (C, 256), F32, kind="ExternalInput").ap()
    wo_d = nc.dram_tensor("woT", (128, 128), F32, kind="ExternalInput").ap()
    y_d = nc.dram_tensor("y", (C, IB), F32, kind="ExternalOutput").ap()

    with tile.TileContext(nc) as tc:
        from contextlib import ExitStack

        with ExitStack() as ctx:
            const = ctx.enter_context(tc.tile_pool(name="const", bufs=1))

            # ---- inputs to SBUF (fp32 staging), convert to bf16 for the PE
            # (fp32 matmuls run fp32_mode=LOW_HIGH = 2x streaming passes, so
            # every PE operand is bf16). DMA order = first-needed-first;
            # casts split across Vector and Scalar to shorten the prologue.
            x1f = const.tile([C, IB], F32)
            nc.sync.dma_start(x1f[:], x1_d[:])
            wqf = const.tile([C, 256], F32)
            nc.sync.dma_start(wqf[:], wq_d[:])
            wkf = const.tile([C, 256], F32)
            nc.sync.dma_start(wkf[:], wk_d[:])
            wvf = const.tile([C, 256], F32)
            nc.sync.dma_start(wvf[:], wv_d[:])
            x2f = const.tile([C, J], F32)
            nc.sync.dma_start(x2f[:, 0:1536], x2_d[:, 0:1536])
            nc.sync.dma_start(x2f[:, 1536:3072], x2_d[:, 1536:3072])
            wof = const.tile([128, 128], F32)
            nc.sync.dma_start(wof[:], wo_d[:])

            x1s = const.tile([C, IB], BF16)
            nc.vector.tensor_copy(x1s[:], x1f[:])
            wqs = const.tile([C, 256], BF16)
            nc.vector.tensor_copy(wqs[:], wqf[:])
            wks = const.tile([C, 256], BF16)
            nc.vector.tensor_copy(wks[:], wkf[:])
            wvs = const.tile([C, 256], BF16)
            nc.vector.tensor_copy(wvs[:], wvf[:])
            x2s = const.tile([C, J], BF16)
            nc.scalar.copy(x2s[:, 0:1536], x2f[:, 0:1536])
            nc.scalar.copy(x2s[:, 1536:3072], x2f[:, 1536:3072])
            wos = const.tile([128, 128], BF16)
            nc.vector.tensor_copy(wos[:], wof[:])

            # ---- persistent SBUF tensors (attention operands in bf16:
            # fp32 matmuls run fp32_mode=LOW_HIGH = 2x streaming passes)
            qts = const.tile([128, 1024], BF16)       # quad q at cols [512q:512q+512]
            kts = [
                const.tile([128, J], BF16, name=f"kt{q}", tag=f"kt{q}")
                for q in range(2)
            ]
            vts = const.tile([128, NT * GSTRIDE], BF16)  # [v | 1] aug, (j, head*33)
            ots_sb = [
                const.tile([128, IB], BF16, name=f"osb{q}", tag=f"osb{q}")
                for q in range(2)
            ]
            ys = const.tile([C, IB], F32)

            # ---- projections (resident 2-bank PSUM pool; pieces are
            # interleaved with the early main loop so the PE stays dense and
            # the exp stream starts ~5us in instead of ~18us)
            ppool = ctx.enter_context(
                tc.tile_pool(name="proj_ps", bufs=1, space="PSUM")
            )

            # ones columns of vts (col 32 of each 33-wide head block)
            ones_v = vts[:].rearrange("p (t h x) -> p t h x", t=NT, x=33)[
                :, :, :, 32:33
            ]
            nc.vector.memset(ones_v, 1.0)

            # PE warmup: ~6us of dense back-to-back matmuls on a zeroed tile
            # while the input DMAs/casts run. The HAM clock gate only
            # un-throttles (1.2 -> 2.4 GHz) after a ~3.4us window of
            # CONTINUOUS PE activity, which the micro-gapped main loop never
            # provides; once warm, the main loop's small gaps keep it warm.
            wrm = const.tile([32, 512], BF16)
            nc.gpsimd.memset(wrm[:], 0.0)
            wp = ppool.tile([128, 1024], F32, tag="proj", name="wp")
            for i in range(10):
                nc.tensor.matmul(
                    wp[0:128, 0:512],
                    wrm[:, 0:128],
                    wrm[:, 0:512],
                    start=True,
                    stop=True,
                    tile_position=(0, 0),
                )

            def emit_qt():
                qp = ppool.tile([128, 1024], F32, tag="proj", name="qp")
                for q in range(2):
                    nc.tensor.matmul(
                        qp[:, 512 * q : 512 * (q + 1)],
                        wqs[:, 128 * q : 128 * (q + 1)],
                        x1s[:],
                        start=True,
                        stop=True,
                    )
                nc.vector.tensor_copy(qts[:], qp[:])

            def emit_kt(q, jb):
                kp = ppool.tile([128, 1024], F32, tag="proj", name="kp")
                for s in range(2):
                    nc.tensor.matmul(
                        kp[:, 512 * s : 512 * (s + 1)],
                        wks[:, 128 * q : 128 * (q + 1)],
                        x2s[:, 1024 * jb + 512 * s : 1024 * jb + 512 * (s + 1)],
                        start=True,
                        stop=True,
                    )
                nc.vector.tensor_copy(kts[q][:, 1024 * jb : 1024 * (jb + 1)], kp[:])

            def emit_vt(tp):
                vp = ppool.tile([128, 1024], F32, tag="proj", name="vp")
                for s in range(4):
                    t = 4 * tp + s
                    nc.tensor.matmul(
                        vp[:, 256 * s : 256 * (s + 1)],
                        x2s[:, 128 * t : 128 * (t + 1)],
                        wvs[:],
                        start=True,
                        stop=True,
                    )
                dst = vts[
                    :, 4 * GSTRIDE * tp : 4 * GSTRIDE * (tp + 1)
                ].rearrange("p (t h x) -> p t h x", t=4, x=33)[:, :, :, 0:32]
                src = vp[:].rearrange("p (t h x) -> p t h x", t=4, x=32)
                nc.vector.tensor_copy(dst, src)

            emit_qt()
            emit_kt(0, 0)
            emit_vt(0)
            # remaining pieces, emitted one per early group of pair 0
            # (deadlines: vt(i) by t=4i, kt(0,jb) by t=8jb, kt(1,*) by pair 2)
            pending = [
                lambda: emit_vt(1),
                lambda: emit_vt(2),
                lambda: emit_kt(0, 1),
                lambda: emit_vt(3),
                lambda: emit_kt(0, 2),
                lambda: emit_vt(4),
                lambda: emit_vt(5),
                lambda: emit_kt(1, 0),
                lambda: emit_kt(1, 1),
                lambda: emit_kt(1, 2),
            ]

            # ---- attention main loop: 2-head groups, double-buffered sim
            # PSUM (2 banks x 2 bufs) + 2 OT accumulator banks (pairs reuse
            # them) + 2 resident proj banks = 8 banks total
            with ExitStack() as mctx:
                otp = mctx.enter_context(
                    tc.tile_pool(name="ot_ps", bufs=1, space="PSUM")
                )
                simp = mctx.enter_context(
                    tc.tile_pool(name="sim_ps", bufs=2, space="PSUM")
                )
                ptsp = mctx.enter_context(tc.tile_pool(name="pts_sb", bufs=3))
                epi = mctx.enter_context(tc.tile_pool(name="epi_sb", bufs=1))

                for p in range(4):
                    q = p // 2
                    otb = otp.tile(
                        [128, IB], F32, tag=f"otb{p % 2}", name=f"otb{p}"
                    )
                    for t in range(NT):
                        if p == 0 and 1 <= t <= len(pending):
                            pending[t - 1]()
                        st = simp.tile([128, 1024], F32, tag="st", name="st")
                        for s in range(2):
                            h = 2 * p + s
                            rl = h % 4
                            nc.tensor.matmul(
                                st[:, 512 * s : 512 * (s + 1)],
                                kts[q][
                                    32 * rl : 32 * (rl + 1),
                                    128 * t : 128 * (t + 1),
                                ],
                                qts[32 * rl : 32 * (rl + 1), 512 * q : 512 * (q + 1)],
                                start=True,
                                stop=True,
                                tile_position=(32 * rl, 0),
                            )
                        pt = ptsp.tile([128, 1024], BF16, tag="pt", name="pt")
                        nc.scalar.activation(pt[:], st[:], ACT_EXP, scale=SCALE)
                        for s in range(2):
                            h = 2 * p + s
                            bp = 64 * s
                            nc.tensor.matmul(
                                otb[bp : bp + 33, :],
                                vts[:, GSTRIDE * t + 33 * h : GSTRIDE * t + 33 * (h + 1)],
                                pt[:, 512 * s : 512 * (s + 1)],
                                start=(t == 0),
                                stop=(t == NT - 1),
                                tile_position=(0, bp),
                                skip_group_check=True,
                            )

                    # epilogue for pair p (overlaps the next pair's main loop)
                    for s in range(2):
                        h = 2 * p + s
                        rl = h % 4
                        bp = 64 * s
                        den = epi.tile([1, IB], F32, tag=f"den{h}", name=f"den{h}")
                        nc.vector.tensor_copy(den[:], otb[bp + 32 : bp + 33, :])
                        rec = epi.tile([1, IB], F32, tag=f"rec{h}", name=f"rec{h}")
                        nc.vector.reciprocal_approx_fast(rec[:], den[:])
                        bca = epi.tile([32, IB], F32, tag=f"bca{h}", name=f"bca{h}")
                        nc.gpsimd.partition_broadcast(bca[:], rec[:], channels=32)
                        nc.vector.tensor_mul(
                            ots_sb[q][32 * rl : 32 * (rl + 1), :],
                            otb[bp : bp + 32, :],
                            bca[:],
                        )

            # ---- final projection y = WoutT.T @ OT (accumulate over quads)
            with tc.tile_pool(name="tail_ps", bufs=1, space="PSUM") as tailp:
                yp = tailp.tile([C, IB], F32)
                for q in range(2):
                    nc.tensor.matmul(
                        yp[:],
                        wos[:, 64 * q : 64 * (q + 1)],
                        ots_sb[q][:],
                        start=(q == 0),
                        stop=(q == 1),
                    )
                nc.vector.tensor_copy(ys[:], yp[:])
            nc.sync.dma_start(y_d[:], ys[:])

    nc.compile()
    return nc


def _prep_core_inputs(x1, x2, Wq, Wkv, Wout):
    x1 = np.asarray(x1, dtype=np.float32)
    x2 = np.asarray(x2, dtype=np.float32)
    Wq = np.asarray(Wq, dtype=np.float32)
    Wkv = np.asarray(Wkv, dtype=np.float32)
    Wout = np.asarray(Wout, dtype=np.float32)

    wqT = np.ascontiguousarray(Wq.T)                      # (64, 256)
    wkT = np.ascontiguousarray(Wkv[:256].T)               # (64, 256)
    wvT = np.ascontiguousarray(Wkv[256:].T)               # (64, 256)
    # WoutT (256, 64) packed as (128, 128): chunk q at cols [64q:64q+64]
    woT = np.ascontiguousarray(
        Wout.T.reshape(2, 128, 64).transpose(1, 0, 2).reshape(128, 128)
    )

    in_maps = []
    for f in range(M):
        x1f = x1[0, f].reshape(C, HWTOK)                          # (64, 1024)
        x2f = np.ascontiguousarray(
            x2[0, f].transpose(1, 0, 2, 3).reshape(C, J)          # (64, 3072)
        )
        for half in range(2):
            in_maps.append(
                {
                    "x1c": np.ascontiguousarray(x1f[:, IB * half : IB * (half + 1)]),
                    "x2c": x2f,
                    "wqT": wqT,
                    "wkT": wkT,
                    "wvT": wvT,
                    "woT": woT,
                }
            )
    return in_maps


def kernel(x1, x2, Wq, Wkv, Wout):
    from concourse.bass_utils import run_bass_kernel_spmd

    if "nc" not in _CACHE:
        _CACHE["nc"] = _build_nc()
    nc = _CACHE["nc"]

    in_maps = _prep_core_inputs(x1, x2, Wq, Wkv, Wout)
    res = run_bass_kernel_spmd(nc, in_maps, core_ids=list(range(8)))

    out = np.empty((B, M, C, H, W), dtype=np.float32)
    for f in range(M):
        yf = np.empty((C, HWTOK), dtype=np.float32)
        for half in range(2):
            yf[:, IB * half : IB * (half + 1)] = res.results[2 * f + half]["y"]
        out[0, f] = yf.reshape(C, H, W)
    return out


# revision 15
# speedup vs baseline: 2.5468x; 1.0033x over previous
"""Trainium2 Bass kernel for nn_CrossAttention (b,m,c,H,W cross-attention).

Problem (hardcoded shapes): b=1, m=4, n=3, c=64, H=W=32, heads=8, dim_head=32.

  q  = Wq  @ x1   per frame        (256, 1024)
  kv = Wkv @ x2   per frame        (512, 3072)
  per (frame, head): attn softmax((q k^T)/sqrt(d)) @ v,  d=32
  y  = Wout @ out  per frame       (64, 1024)

Sharding: 8 cores = 4 frames x 2 q-token halves. Each core gets all 8 heads,
512 q tokens, the full 3072 kv tokens of its frame. No cross-core comms;
outputs concatenate.

Per-core kernel layout strategy:
  - QT (256, 512) / KT (256, 3072) channel-major from 1x1-conv projections,
    heads at 32-partition offsets (quad tiles of 4 heads x 32 dims = 128).
  - scores computed TRANSPOSED: ST[j, i] = sum_d k[j,d] q[i,d] via PE
    row-tiling (4 heads concurrently, K=32 each at tile_position (32r, 0)).
  - softmax without max-subtraction (scores are bounded ~ +-1 for this
    problem's scaling) -> single ACT Exp pass PSUM->SBUF, FD=2048.
  - second matmul consumes exp(ST) directly as the moving operand with
    stationary [v | 1] (128, 33): row 32 accumulates the softmax denominator
    for free. Two heads share a PSUM bank via column-tiling (0 / 64).
  - normalize: gather denominators, reciprocal_approx_fast, gpsimd
    partition_broadcast, tensor_mul into SBUF.
  - final projection Y = Wout @ OT accumulated over the two head-quads.
"""

import numpy as np

B, M, N_CTX, C, H, W = 1, 4, 3, 64, 32, 32
HEADS, D = 8, 32
HWTOK = H * W          # 1024 tokens per frame
IB = 512               # q tokens per core
J = N_CTX * HWTOK      # 3072 kv tokens
NT = J // 128          # 24 j-tiles
GSTRIDE = 33 * HEADS   # 264: aug stride per j-tile in vts
SCALE = float(D) ** -0.5

_CACHE = {}


def _build_nc():
    import concourse.tile as tile
    from concourse import bacc, mybir

    F32 = mybir.dt.float32
    BF16 = mybir.dt.bfloat16
    ACT_EXP = mybir.ActivationFunctionType.Exp

    nc = bacc.Bacc(
        "TRN2",
        target_bir_lowering=False,
        debug=False,
        enable_asserts=True,
        num_devices=8,
    )

    x1_d = nc.dram_tensor("x1c", (C, IB), F32, kind="ExternalInput").ap()
    x2_d = nc.dram_tensor("x2c", (C, J), F32, kind="ExternalInput").ap()
    wq_d = nc.dram_tensor("wqT", (C, 256), F32, kind="ExternalInput").ap()
    wk_d = nc.dram_tensor("wkT", (C, 256), F32, kind="ExternalInput").ap()
    wv_d = nc.dram_tensor("wvT", (C, 256), F32, kind="ExternalInput").ap()
    wo_d = nc.dram_tensor("woT", (128, 128), F32, kind="ExternalInput").ap()
    y_d = nc.dram_tensor("y", (C, IB), F32, kind="ExternalOutput").ap()

    with tile.TileContext(nc) as tc:
        from contextlib import ExitStack

        with ExitStack() as ctx:
            const = ctx.enter_context(tc.tile_pool(name="const", bufs=1))

            # ---- inputs to SBUF (fp32 staging), convert to bf16 for the PE
            # (fp32 matmuls run fp32_mode=LOW_HIGH = 2x streaming passes, so
            # every PE operand is bf16). DMA order = first-needed-first;
            # casts split across Vector and Scalar to shorten the prologue.
            x1f = const.tile([C, IB], F32)
            nc.sync.dma_start(x1f[:], x1_d[:])
            wqf = const.tile([C, 256], F32)
            nc.sync.dma_start(wqf[:], wq_d[:])
            wkf = const.tile([C, 256], F32)
            nc.sync.dma_start(wkf[:], wk_d[:])
            wvf = const.tile([C, 256], F32)
            nc.sync.dma_start(wvf[:], wv_d[:])
            x2f = const.tile([C, J], F32)
            nc.sync.dma_start(x2f[:, 0:1536], x2_d[:, 0:1536])
            nc.sync.dma_start(x2f[:, 1536:3072], x2_d[:, 1536:3072])
            wof = const.tile([128, 128], F32)
            nc.sync.dma_start(wof[:], wo_d[:])

            x1s = const.tile([C, IB], BF16)
            nc.vector.tensor_copy(x1s[:], x1f[:])
            wqs = const.tile([C, 256], BF16)
            nc.vector.tensor_copy(wqs[:], wqf[:])
            wks = const.tile([C, 256], BF16)
            nc.vector.tensor_copy(wks[:], wkf[:])
            wvs = const.tile([C, 256], BF16)
            nc.vector.tensor_copy(wvs[:], wvf[:])
            x2s = const.tile([C, J], BF16)
            nc.scalar.copy(x2s[:, 0:1536], x2f[:, 0:1536])
            nc.scalar.copy(x2s[:, 1536:3072], x2f[:, 1536:3072])
            wos = const.tile([128, 128], BF16)
            nc.vector.tensor_copy(wos[:], wof[:])

            # ---- persistent SBUF tensors (attention operands in bf16:
            # fp32 matmuls run fp32_mode=LOW_HIGH = 2x streaming passes)
            qts = const.tile([128, 1024], BF16)       # quad q at cols [512q:512q+512]
            kts = [
                const.tile([128, J], BF16, name=f"kt{q}", tag=f"kt{q}")
                for q in range(2)
            ]
            vts = const.tile([128, NT * GSTRIDE], BF16)  # [v | 1] aug, (j, head*33)
            ots_sb = [
                const.tile([128, IB], BF16, name=f"osb{q}", tag=f"osb{q}")
                for q in range(2)
            ]
            ys = const.tile([C, IB], F32)

            # ---- projections (resident 2-bank PSUM pool; pieces are
            # interleaved with the early main loop so the PE stays dense and
            # the exp stream starts ~5us in instead of ~18us)
            ppool = ctx.enter_context(
                tc.tile_pool(name="proj_ps", bufs=1, space="PSUM")
            )

            # ones columns of vts (col 32 of each 33-wide head block)
            ones_v = vts[:].rearrange("p (t h x) -> p t h x", t=NT, x=33)[
                :, :, :, 32:33
            ]
            nc.vector.memset(ones_v, 1.0)

            # PE warmup: ~6us of dense back-to-back matmuls on a zeroed tile
            # while the input DMAs/casts run. The HAM clock gate only
            # un-throttles (1.2 -> 2.4 GHz) after a ~3.4us window of
            # CONTINUOUS PE activity, which the micro-gapped main loop never
            # provides; once warm, the main loop's small gaps keep it warm.
            wrm = const.tile([32, 512], BF16)
            nc.gpsimd.memset(wrm[:], 0.0)
            wp = ppool.tile([128, 1024], F32, tag="proj", name="wp")
            for i in range(10):
                nc.tensor.matmul(
                    wp[0:128, 0:512],
                    wrm[:, 0:128],
                    wrm[:, 0:512],
                    start=True,
                    stop=True,
                    tile_position=(0, 0),
                )

            def emit_qt():
                qp = ppool.tile([128, 1024], F32, tag="proj", name="qp")
                for q in range(2):
                    nc.tensor.matmul(
                        qp[:, 512 * q : 512 * (q + 1)],
                        wqs[:, 128 * q : 128 * (q + 1)],
                        x1s[:],
                        start=True,
                        stop=True,
                    )
                nc.vector.tensor_copy(qts[:], qp[:])

            def emit_kt(q, jb):
                kp = ppool.tile([128, 1024], F32, tag="proj", name="kp")
                for s in range(2):
                    nc.tensor.matmul(
                        kp[:, 512 * s : 512 * (s + 1)],
                        wks[:, 128 * q : 128 * (q + 1)],
                        x2s[:, 1024 * jb + 512 * s : 1024 * jb + 512 * (s + 1)],
                        start=True,
                        stop=True,
                    )
                nc.vector.tensor_copy(kts[q][:, 1024 * jb : 1024 * (jb + 1)], kp[:])

            def emit_vt(tp):
                vp = ppool.tile([128, 1024], F32, tag="proj", name="vp")
                for s in range(4):
                    t = 4 * tp + s
                    nc.tensor.matmul(
                        vp[:, 256 * s : 256 * (s + 1)],
                        x2s[:, 128 * t : 128 * (t + 1)],
                        wvs[:],
                        start=True,
                        stop=True,
                    )
                dst = vts[
                    :, 4 * GSTRIDE * tp : 4 * GSTRIDE * (tp + 1)
                ].rearrange("p (t h x) -> p t h x", t=4, x=33)[:, :, :, 0:32]
                src = vp[:].rearrange("p (t h x) -> p t h x", t=4, x=32)
                nc.vector.tensor_copy(dst, src)

            emit_qt()
            emit_kt(0, 0)
            emit_vt(0)
            # remaining pieces, emitted one per early group of pair 0
            # (deadlines: vt(i) by t=4i, kt(0,jb) by t=8jb, kt(1,*) by pair 2)
            pending = [
                lambda: emit_vt(1),
                lambda: emit_vt(2),
                lambda: emit_kt(0, 1),
                lambda: emit_vt(3),
                lambda: emit_kt(0, 2),
                lambda: emit_vt(4),
                lambda: emit_vt(5),
                lambda: emit_kt(1, 0),
                lambda: emit_kt(1, 1),
                lambda: emit_kt(1, 2),
            ]

            # ---- attention main loop: 2-head groups, double-buffered sim
            # PSUM (2 banks x 2 bufs) + 2 OT accumulator banks (pairs reuse
            # them) + 2 resident proj banks = 8 banks total
            with ExitStack() as mctx:
                otp = mctx.enter_context(
                    tc.tile_pool(name="ot_ps", bufs=1, space="PSUM")
                )
                simp = mctx.enter_context(
                    tc.tile_pool(name="sim_ps", bufs=2, space="PSUM")
                )
                ptsp = mctx.enter_context(tc.tile_pool(name="pts_sb", bufs=4))
                epi = mctx.enter_context(tc.tile_pool(name="epi_sb", bufs=1))

                for p in range(4):
                    q = p // 2
                    otb = otp.tile(
                        [128, IB], F32, tag=f"otb{p % 2}", name=f"otb{p}"
                    )
                    for t in range(NT):
                        if p == 0 and 1 <= t <= len(pending):
                            pending[t - 1]()
                        st = simp.tile([128, 1024], F32, tag="st", name="st")
                        for s in range(2):
                            h = 2 * p + s
                            rl = h % 4
                            nc.tensor.matmul(
                                st[:, 512 * s : 512 * (s + 1)],
                                kts[q][
                                    32 * rl : 32 * (rl + 1),
                                    128 * t : 128 * (t + 1),
                                ],
                                qts[32 * rl : 32 * (rl + 1), 512 * q : 512 * (q + 1)],
                                start=True,
                                stop=True,
                                tile_position=(32 * rl, 0),
                            )
                        pt = ptsp.tile([128, 1024], BF16, tag="pt", name="pt")
                        nc.scalar.activation(pt[:], st[:], ACT_EXP, scale=SCALE)
                        for s in range(2):
                            h = 2 * p + s
                            bp = 64 * s
                            nc.tensor.matmul(
                                otb[bp : bp + 33, :],
                                vts[:, GSTRIDE * t + 33 * h : GSTRIDE * t + 33 * (h + 1)],
                                pt[:, 512 * s : 512 * (s + 1)],
                                start=(t == 0),
                                stop=(t == NT - 1),
                                tile_position=(0, bp),
                                skip_group_check=True,
                            )

                    # epilogue for pair p (overlaps the next pair's main loop)
                    for s in range(2):
                        h = 2 * p + s
                        rl = h % 4
                        bp = 64 * s
                        den = epi.tile([1, IB], F32, tag=f"den{h}", name=f"den{h}")
                        nc.vector.tensor_copy(den[:], otb[bp + 32 : bp + 33, :])
                        rec = epi.tile([1, IB], F32, tag=f"rec{h}", name=f"rec{h}")
                        nc.vector.reciprocal_approx_fast(rec[:], den[:])
                        bca = epi.tile([32, IB], F32, tag=f"bca{h}", name=f"bca{h}")
                        nc.gpsimd.partition_broadcast(bca[:], rec[:], channels=32)
                        nc.vector.tensor_mul(
                            ots_sb[q][32 * rl : 32 * (rl + 1), :],
                            otb[bp : bp + 32, :],
                            bca[:],
                        )

            # ---- final projection y = WoutT.T @ OT (accumulate over quads)
            with tc.tile_pool(name="tail_ps", bufs=1, space="PSUM") as tailp:
                yp = tailp.tile([C, IB], F32)
                for q in range(2):
                    nc.tensor.matmul(
                        yp[:],
                        wos[:, 64 * q : 64 * (q + 1)],
                        ots_sb[q][:],
                        start=(q == 0),
                        stop=(q == 1),
                    )
                nc.vector.tensor_copy(ys[:], yp[:])
            nc.sync.dma_start(y_d[:], ys[:])

    nc.compile()
    return nc


def _prep_core_inputs(x1, x2, Wq, Wkv, Wout):
    x1 = np.asarray(x1, dtype=np.float32)
    x2 = np.asarray(x2, dtype=np.float32)
    Wq = np.asarray(Wq, dtype=np.float32)
    Wkv = np.asarray(Wkv, dtype=np.float32)
    Wout = np.asarray(Wout, dtype=np.float32)

    wqT = np.ascontiguousarray(Wq.T)                      # (64, 256)
    wkT = np.ascontiguousarray(Wkv[:256].T)               # (64, 256)
    wvT = np.ascontiguousarray(Wkv[256:].T)               # (64, 256)
    # WoutT (256, 64) packed as (128, 128): chunk q at cols [64q:64q+64]
    woT = np.ascontiguousarray(
        Wout.T.reshape(2, 128, 64).transpose(1, 0, 2).reshape(128, 128)
    )

    in_maps = []
    for f in range(M):
        x1f = x1[0, f].reshape(C, HWTOK)                          # (64, 1024)
        x2f = np.ascontiguousarray(
            x2[0, f].transpose(1, 0, 2, 3).reshape(C, J)          # (64, 3072)
        )
        for half in range(2):
            in_maps.append(
                {
                    "x1c": np.ascontiguousarray(x1f[:, IB * half : IB * (half + 1)]),
                    "x2c": x2f,
                    "wqT": wqT,
                    "wkT": wkT,
                    "wvT": wvT,
                    "woT": woT,
                }
            )
    return in_maps


def kernel(x1, x2, Wq, Wkv, Wout):
    from concourse.bass_utils import run_bass_kernel_spmd

    if "nc" not in _CACHE:
        _CACHE["nc"] = _build_nc()
    nc = _CACHE["nc"]

    in_maps = _prep_core_inputs(x1, x2, Wq, Wkv, Wout)
    res = run_bass_kernel_spmd(nc, in_maps, core_ids=list(range(8)))

    out = np.empty((B, M, C, H, W), dtype=np.float32)
    for f in range(M):
        yf = np.empty((C, HWTOK), dtype=np.float32)
        for half in range(2):
            yf[:, IB * half : IB * (half + 1)] = res.results[2 * f + half]["y"]
        out[0, f] = yf.reshape(C, H, W)
    return out
